# revision 1
# baseline (speedup 1.0000x reference)
"""Sharded MHA-with-RoPE Trainium2 kernel (nn_CustomTorchMHASelf).

Contract: kernel(**inputs) takes the FULL unsharded inputs of the
reference (x [2,2048,2048], Wqkv_w [6144,2048], Wqkv_b [6144],
out_w [2048,2048], out_b [2048]) and returns the full [2,2048,2048]
fp32 output, running the compute on 8 NeuronCores.

Sharding: core = b*4 + g handles batch b and head-group g (4 of the 16
heads). Each core computes q/k/v projections for its heads, RoPE,
softmax attention, and its slice of the out-projection; the host sums
the 4 partial outputs per batch and adds out_b.

Device data plane is bf16 (fp32 PSUM accumulation); the host
pre-transposes x and the weight slices into the layouts the TensorE
wants (contraction dim on partitions everywhere).
"""

import math
import os
import sys
import types

import numpy as np
import ml_dtypes

import concourse.bass as bass
import concourse.mybir as mybir
import concourse.tile as tile
from concourse.bass import ds

F32 = mybir.dt.float32
BF16 = mybir.dt.bfloat16
Alu = mybir.AluOpType
Act = mybir.ActivationFunctionType
BF = ml_dtypes.bfloat16

S, E, HTOT, HL, D, P = 2048, 2048, 16, 4, 128, 128

# Filled with the profile exec time (ns) when MHA_TRACE=1; read by test.py.
LAST_EXEC_NS = None


def _install_axon_ntff_shim():
    """Provide antenv.axon_hooks so trace=True can reach the axon NTFF hook."""
    if "antenv.axon_hooks" in sys.modules:
        return
    mod = types.ModuleType("antenv.axon_hooks")
    holder = [None]
    mod.set_axon_ntff_profile_hook = lambda h: holder.__setitem__(0, h)
    mod.get_axon_ntff_profile_hook = lambda: holder[0]
    sys.modules["antenv.axon_hooks"] = mod
    try:
        import antenv
        antenv.axon_hooks = mod
    except ImportError:
        pass
    # boot() ran at interpreter start (sitecustomize), before this module
    # existed, so its NTFF-hook registration was silently skipped. Redo it.
    try:
        from trn_agent_boot.trn_boot import _ntff_profile_via_ctypes
        hook = _ntff_profile_via_ctypes("/opt/axon/libaxon_pjrt.so")
        if hook is not None:
            mod.set_axon_ntff_profile_hook(hook)
    except Exception:
        pass


def _split_multi_waits(nc):
    """Hoist extra sem-waits onto standalone NoOps (one wait per inst).

    This walrus build rejects any instruction carrying more than one
    sync-wait ("Too many sync wait commands"); Tile attaches one wait per
    outstanding semaphore to the consuming instruction. Splitting them
    across same-engine NoOps placed immediately before is equivalent:
    the engine executes serially, so all waits still precede the inst.
    """
    ctr = 0
    for fn in nc.m.functions:
        for blk in fn.blocks:
            out = []
            for inst in blk.instructions:
                si = getattr(inst, "sync_info", None)
                if si is not None and si.on_wait is not None                         and len(si.on_wait) > 1:
                    waits = list(si.on_wait)
                    si.on_wait = [waits[-1]]
                    for w in waits[:-1]:
                        ctr += 1
                        nop = mybir.InstNoOp(
                            name=f"I-wsplit-{ctr}", ins=[], outs=[])
                        nop.engine = inst.engine
                        nop.sync_info = mybir.SyncInfo(
                            on_wait=[w], on_update=[])
                        out.append(nop)
                out.append(inst)
            blk.instructions[:] = out


def _build_mha(nc: bass.Bass):
    """Emit the per-core MHA program (one shard) into `nc`."""
    EO = E // P            # contraction subtiles for the projections
    NQK = 2 * HL           # q/k feature blocks
    ST = 512               # free-dim tile (one PSUM bank of fp32)
    NS = S // ST
    SB = S // P
    JT = S // P            # key blocks per head
    ET = E // ST
    H = D // 2

    xT = nc.dram_tensor("xT", [E, S], BF16, kind="ExternalInput")
    wqkT = nc.dram_tensor("wqkT", [E, NQK * D], BF16, kind="ExternalInput")
    wvT = nc.dram_tensor("wvT", [E, HL * D], BF16, kind="ExternalInput")
    qkb = nc.dram_tensor("qkb", [NQK, D], F32, kind="ExternalInput")
    vb = nc.dram_tensor("vb", [HL * D], F32, kind="ExternalInput")
    cosT = nc.dram_tensor("cosT", [D, S], F32, kind="ExternalInput")
    sinT = nc.dram_tensor("sinT", [D, S], F32, kind="ExternalInput")
    owT = nc.dram_tensor("owT", [HL * D, E], BF16, kind="ExternalInput")
    ones = nc.dram_tensor("ones", [P, P], BF16, kind="ExternalInput")
    out = nc.dram_tensor("out", [S, E], F32, kind="ExternalOutput")

    isc = 1.0 / math.sqrt(D)

    from contextlib import ExitStack

    with tile.TileContext(nc) as tc, ExitStack() as stk:
        persist = stk.enter_context(tc.tile_pool(name="persist", bufs=1))
        qkT_sb = persist.tile([P, NQK, S], BF16)    # q/k post-RoPE [d, jb, s]
        v_sb = persist.tile([P, SB, HL * D], BF16)  # v natural [s%128, s//128, hd]
        ones_sb = persist.tile([P, P], BF16)
        nc.sync.dma_start(ones_sb[:], ones[:])
        bctx = stk.enter_context(tc.tile_pool(name="bctx", bufs=1))

        psA = stk.enter_context(tc.tile_pool(name="psA", bufs=4, space="PSUM"))
        psS = stk.enter_context(tc.tile_pool(name="psS", bufs=4, space="PSUM"))

        # ---- Phase A: QKV projection + bias + RoPE ----
        with tc.tile_pool(name="phaseA", bufs=1) as pa, \
             tc.tile_pool(name="xstream", bufs=2) as xs, \
             tc.tile_pool(name="ropetmp", bufs=2) as rt:
            cos_sb = pa.tile([P, S], F32)
            sin_sb = pa.tile([P, S], F32)
            nc.sync.dma_start(cos_sb[:], cosT[:])
            nc.sync.dma_start(sin_sb[:], sinT[:])
            qkb_sb = pa.tile([P, NQK], F32)
            nc.sync.dma_start(qkb_sb[:], qkb[:].rearrange("c d -> d c"))
            vb_sb = pa.tile([P, HL * D], F32)
            nc.sync.dma_start(vb_sb[:], vb[None, :].to_broadcast((P, HL * D)))
            wqk_sb = pa.tile([P, EO, NQK * D], BF16)
            wv_sb = pa.tile([P, EO, HL * D], BF16)
            # interleave the first x-slice with the weights so the first
            # matmuls (which consume eo=0 tiles) aren't queued behind all
            # 12 MB of weight DMA
            xt0 = xs.tile([P, EO, ST], BF16, tag="xt", name="xt0")
            for eo in range(EO):
                nc.sync.dma_start(wqk_sb[:, eo, :], wqkT[ds(eo * P, P), :])
                nc.sync.dma_start(xt0[:, eo, :], xT[ds(eo * P, P), ds(0, ST)])
                nc.sync.dma_start(wv_sb[:, eo, :], wvT[ds(eo * P, P), :])

            for i in range(NS):
                if i == 0:
                    xt = xt0
                else:
                    xt = xs.tile([P, EO, ST], BF16, tag="xt")
                    for eo in range(EO):
                        nc.sync.dma_start(
                            xt[:, eo, :], xT[ds(eo * P, P), ds(i * ST, ST)])
                sl = ds(i * ST, ST)
                for jb in range(NQK):
                    ps = psA.tile([P, ST], F32, tag="acc")
                    for eo in range(EO):
                        nc.tensor.matmul(
                            ps[:], wqk_sb[:, eo, ds(jb * D, D)], xt[:, eo, :],
                            start=(eo == 0), stop=(eo == EO - 1))
                    # RoPE: qb = q + bias; rot = half-swap(qb) via DMA
                    # (cross-partition moves need DMA); out = qb*cos +
                    # rot*sinSW with the rotation sign folded into the
                    # host-prepped sin table.
                    qb = rt.tile([P, ST], F32, tag="qb")
                    nc.vector.tensor_scalar_add(
                        qb[:], ps[:], qkb_sb[:, jb, None])
                    rot = rt.tile([P, ST], F32, tag="rot")
                    nc.sync.dma_start(rot[:H], qb[H:])
                    nc.sync.dma_start(rot[H:], qb[:H])
                    t1 = rt.tile([P, ST], F32, tag="t1")
                    t2 = rt.tile([P, ST], F32, tag="t2")
                    nc.vector.tensor_tensor(
                        t1[:], qb[:], cos_sb[:, sl], Alu.mult)
                    nc.vector.tensor_tensor(
                        t2[:], rot[:], sin_sb[:, sl], Alu.mult)
                    nc.vector.tensor_tensor(
                        qkT_sb[:, jb, sl], t1[:], t2[:], Alu.add)
                for sbl in range(ST // P):
                    sb = i * (ST // P) + sbl
                    ps = psA.tile([P, ST], F32, tag="acc")
                    for eo in range(EO):
                        nc.tensor.matmul(
                            ps[:, : HL * D], xt[:, eo, ds(sbl * P, P)],
                            wv_sb[:, eo, :], start=(eo == 0), stop=(eo == EO - 1))
                    nc.vector.tensor_tensor(
                        v_sb[:, sb, :], ps[:, : HL * D], vb_sb[:], Alu.add)

        # ---- Phase B: attention per head ----
        ctxT_sb = bctx.tile([P, HL, S], BF16)       # [d, h, i]
        F32R = mybir.dt.float32r
        with tc.tile_pool(name="phaseB", bufs=2) as pb, \
             tc.tile_pool(name="recipp", bufs=2) as rp:
            for h in range(HL):
                qT_h = qkT_sb[:, 2 * h, :]
                kT_h = qkT_sb[:, 2 * h + 1, :]
                for i in range(NS):
                    att = pb.tile([P, JT, ST], BF16, tag="att")
                    for jb in range(JT):
                        ps = psS.tile([P, ST], F32, tag="sc")
                        nc.tensor.matmul(
                            ps[:], kT_h[:, ds(jb * P, P)],
                            qT_h[:, ds(i * ST, ST)], start=True, stop=True)
                        nc.scalar.activation(
                            att[:, jb, :], ps[:], Act.Exp, scale=isc)
                    psc = psA.tile([P, ST], F32, tag="acc")
                    psd = psA.tile([P, ST], F32, tag="acc")
                    for jb in range(JT):
                        nc.tensor.matmul(
                            psc[:], v_sb[:, jb, ds(h * D, D)], att[:, jb, :],
                            start=(jb == 0), stop=(jb == JT - 1))
                        nc.tensor.matmul(
                            psd[:], ones_sb[:], att[:, jb, :],
                            start=(jb == 0), stop=(jb == JT - 1))
                    rec = rp.tile([P, ST], F32, tag="rec")
                    nc.vector.reciprocal(rec[:], psd[:])
                    nc.vector.tensor_tensor(
                        ctxT_sb[:, h, ds(i * ST, ST)], psc[:], rec[:], Alu.mult)

        # ---- Phase C: out projection ----
        with tc.tile_pool(name="phaseC", bufs=1) as pc, \
             tc.tile_pool(name="ocopy", bufs=4) as oc:
            ow_sb = pc.tile([P, HL, E], BF16)
            for ho in range(HL):
                nc.sync.dma_start(ow_sb[:, ho, :], owT[ds(ho * P, P), :])
            for sb in range(SB):
                for et in range(ET):
                    ps = psA.tile([P, ST], F32, tag="acc")
                    for ho in range(HL):
                        nc.tensor.matmul(
                            ps[:], ctxT_sb[:, ho, ds(sb * P, P)],
                            ow_sb[:, ho, ds(et * ST, ST)],
                            start=(ho == 0), stop=(ho == HL - 1))
                    ot = oc.tile([P, ST], F32, tag="ot")
                    nc.any.tensor_copy(ot[:], ps[:])
                    nc.sync.dma_start(
                        out[ds(sb * P, P), ds(et * ST, ST)], ot[:])

    return nc


def _rope_tables():
    inv_freq = 1.0 / (10000.0 ** (np.arange(0, D, 2, dtype=np.float32) / D))
    t = np.arange(S, dtype=np.float32)
    freqs = np.einsum("s,f->sf", t, inv_freq)
    emb = np.concatenate([freqs, freqs], axis=-1)
    cosT = np.cos(emb).astype(np.float32).T.copy()
    sinT = np.sin(emb).astype(np.float32).T.copy()
    # fold the rotate-half sign in: out = qb*cos + halfswap(qb)*sinSW
    sinSW = np.concatenate([-sinT[:D // 2], sinT[D // 2:]], axis=0)
    return cosT, np.ascontiguousarray(sinSW)


def _core_inputs(x, Wqkv_w, Wqkv_b, out_w, b, g, cosT, sinT, xT_bf):
    qk_cols, qkb_rows = [], []
    for hl in range(HL):
        h = g * HL + hl
        qk_cols.append(Wqkv_w[h * D:(h + 1) * D, :].T)
        qk_cols.append(Wqkv_w[E + h * D:E + (h + 1) * D, :].T)
        qkb_rows.append(Wqkv_b[h * D:(h + 1) * D])
        qkb_rows.append(Wqkv_b[E + h * D:E + (h + 1) * D])
    wqkT = np.ascontiguousarray(np.concatenate(qk_cols, axis=1)).astype(BF)
    qkb = np.stack(qkb_rows).astype(np.float32)
    v0 = 2 * E + g * HL * D
    wvT = np.ascontiguousarray(Wqkv_w[v0:v0 + HL * D, :].T).astype(BF)
    vb = Wqkv_b[v0:v0 + HL * D].astype(np.float32)
    owT = np.ascontiguousarray(
        out_w[:, g * HL * D:(g + 1) * HL * D].T).astype(BF)
    return {"xT": xT_bf, "wqkT": wqkT, "wvT": wvT, "qkb": qkb, "vb": vb,
            "cosT": cosT, "sinT": sinT, "owT": owT,
            "ones": np.ones((P, P), BF)}


def kernel(x, Wqkv_w, Wqkv_b, out_w, out_b):
    global LAST_EXEC_NS
    _install_axon_ntff_shim()
    from concourse.bass_utils import run_bass_kernel_spmd

    x = np.asarray(x, dtype=np.float32)
    Wqkv_w = np.asarray(Wqkv_w, dtype=np.float32)
    Wqkv_b = np.asarray(Wqkv_b, dtype=np.float32)
    out_w = np.asarray(out_w, dtype=np.float32)
    out_b = np.asarray(out_b, dtype=np.float32)

    cosT, sinT = _rope_tables()
    xT_bf = [np.ascontiguousarray(x[b].T).astype(BF) for b in range(2)]
    in_maps = []
    for core in range(8):
        b, g = core // 4, core % 4
        in_maps.append(
            _core_inputs(x, Wqkv_w, Wqkv_b, out_w, b, g, cosT, sinT, xT_bf[b]))

    nc = bass.Bass()
    _build_mha(nc)
    _split_multi_waits(nc)

    trace = bool(os.environ.get("MHA_TRACE"))
    if trace:
        # dev-only profiling path; skip the S3 artifact upload
        import concourse.bass_utils as _bu
        _bu.upload_artifacts = lambda tmpdir: tmpdir
    res = run_bass_kernel_spmd(
        nc, in_maps, core_ids=list(range(8)), trace=trace)
    if trace:
        LAST_EXEC_NS = res.exec_time_ns

    out = np.empty((2, S, E), dtype=np.float32)
    for b in range(2):
        acc = res.results[b * 4 + 0]["out"].astype(np.float32).copy()
        for g in range(1, 4):
            acc += res.results[b * 4 + g]["out"]
        out[b] = acc + out_b[None, :]
    return out

